# revision 10
# baseline (speedup 1.0000x reference)
"""Multi-head attention Trainium2 kernel (Bass/Tile), data-parallel over batch.

Problem shapes (hardcoded): x [8, 1024, 1024] fp32, 16 heads x 64 dim,
shared per-head projections Wq/Wk/Wv [64, 64], output proj Wo [1024, 1024].

Reference math (note quirks):
  xh = x reshaped to [h, b, m, d]
  Q/K/V = xh @ W{q,k,v}.T + b
  scores = einsum('hbmd,hbnd->hbmn', K, Q) / sqrt(1024)   (K @ Q^T!)
  A = softmax(scores, axis=-1)
  out = (A @ V) transposed (0,1,3,2) then .reshape(b, m, D) @ Wo.T + bo

Per-core plan (core b handles batch b, no collectives):
  - host prepares xT = x[b].T, blockdiag lhsT weights for 2-head packed
    projections, WoT = Wo.T
  - QT/KT/VT [64*16, m] via blockdiag [128,128] matmuls
  - scores: per (pair, m-half, n-block) ONE psS tile [128, e-512|o-512];
    the two K=64 matmuls (partitions 0:64 / 64:128) row-tile and run
    CONCURRENTLY on the PE; one ACT instruction exps both.
  - U[65, m] = [V | ones].T @ expS  -> row 64 = softmax denominator
  - PE-transpose U -> [m, 65], normalize cols by 1/denominator -> P.T
    (norm done per m-half right after its attention phase)
  - Y rows for the pair's heads = P.T chunk.T @ WoT (bo added on host)
  - QKV of pair t+1 and output-projection of pair t-1 are emitted as
    fine-grained "fillers" between attention steps so the scheduler can
    use PE slack without ever blocking the score->exp pipeline.
"""

import os

import numpy as np

B = 8
M = 1024
D = 1024
NT = 8  # 128-row tiles in M / D

DTYPE_MODE = os.environ.get("KERNEL_DTYPE", "f16")

DEFAULT_CFG = dict(
    s_bufs=2,             # score psum tiles [128,1024] (2 banks each)
    w_bufs=1,             # weights-path psum tile [128,512]
    y_bufs=1,             # final-path psum tile [128,512]
    u_bufs=2,             # AV accumulator psum tiles [65,512]
    qkv_bufs=3,           # qT/kT/vT sbuf pipelining depth
    vnat_bufs=3,
    usb_bufs=4,
    ysb_bufs=3,
    es_bufs=3,            # expS [128, NT*1024] fp16 tiles
)

_compiled = {}


def _build(mode, cfg=None):
    import concourse.bacc as bacc
    import concourse.mybir as mybir
    import concourse.tile as tile
    from concourse.masks import make_identity

    cfg = dict(DEFAULT_CFG, **(cfg or {}))
    f32 = mybir.dt.float32
    mdt = mybir.dt.float16
    tdt = mdt  # transpose-path dtype
    Exp = mybir.ActivationFunctionType.Exp

    nc = bacc.Bacc("TRN2", target_bir_lowering=False, debug=False, num_devices=B)

    xT_ap = nc.dram_tensor("xT", [D, M], mdt, kind="ExternalInput").ap()
    woT_ap = nc.dram_tensor("woT", [D, D], mdt, kind="ExternalInput").ap()
    wq_ap = nc.dram_tensor("wq", [128, 128], mdt, kind="ExternalInput").ap()
    wk_ap = nc.dram_tensor("wk", [128, 128], mdt, kind="ExternalInput").ap()
    wv_ap = nc.dram_tensor("wv", [128, 128], mdt, kind="ExternalInput").ap()
    bias_ap = nc.dram_tensor("bias", [128, 3], f32, kind="ExternalInput").ap()
    y_ap = nc.dram_tensor("y", [D, M], mdt, kind="ExternalOutput").ap()

    with tile.TileContext(nc) as tc:
        with (
            tc.tile_pool(name="persist", bufs=1) as persist,
            tc.tile_pool(name="qkv", bufs=cfg["qkv_bufs"]) as qkv_pool,
            tc.tile_pool(name="vnat", bufs=cfg.get("vnat_bufs", 2)) as vnat_pool,
            tc.tile_pool(name="exps", bufs=cfg["es_bufs"]) as exps_pool,
            tc.tile_pool(name="usb", bufs=cfg.get("usb_bufs", 3)) as usb_pool,
            tc.tile_pool(name="ysb", bufs=cfg.get("ysb_bufs", 2)) as ysb_pool,
            tc.tile_pool(name="rec", bufs=8) as rec_pool,
            tc.tile_pool(name="ps", bufs=1, space="PSUM") as ps_pool,
        ):
            # ---- persistent tiles ----
            xT_all = persist.tile([128, NT * M], mdt)  # tile t at cols t*M
            woT_all = persist.tile([128, NT * D], mdt)
            PT_all = persist.tile([128, NT * D], mdt)  # [m-local, mt*D + h*64+d]
            wq_sb = persist.tile([128, 128], mdt)
            wk_sb = persist.tile([128, 128], mdt)
            wv_sb = persist.tile([128, 128], mdt)
            bias_sb = persist.tile([128, 3], f32)
            identity = persist.tile([128, 128], tdt)
            dummy = persist.tile([128, 1], f32)

            # Preload the exp table set while input DMAs are in flight.
            nc.scalar.activation(dummy[:], dummy[:], Exp, scale=1.0)

            with nc.named_scope("loads"):
                nc.sync.dma_start(wq_sb[:], wq_ap[:])
                nc.sync.dma_start(wk_sb[:], wk_ap[:])
                nc.sync.dma_start(wv_sb[:], wv_ap[:])
                nc.sync.dma_start(bias_sb[:], bias_ap[:])
                for half in range(2):
                    nc.sync.dma_start(
                        xT_all[:, half * 512 : (half + 1) * 512],
                        xT_ap[0:128, half * 512 : (half + 1) * 512],
                    )
                make_identity(nc, identity[:])
                for t in range(1, NT):
                    for half in range(2):
                        nc.sync.dma_start(
                            xT_all[:, t * M + half * 512 : t * M + (half + 1) * 512],
                            xT_ap[t * 128 : (t + 1) * 128, half * 512 : (half + 1) * 512],
                        )

            def emit_woT_loads():
                with nc.named_scope("loads_wo"):
                    for t in range(NT):
                        nc.sync.dma_start(
                            woT_all[:, t * D : (t + 1) * D],
                            woT_ap[t * 128 : (t + 1) * 128, :],
                        )

            # ---- filler queue: list of 0-arg callables, emitted between
            # attention steps so they sit at lower scheduler priority ----
            fillers = []

            def pop_fillers(k):
                for _ in range(min(k, len(fillers))):
                    fillers.pop(0)()

            def flush_fillers():
                pop_fillers(len(fillers))

            def push_qkv(t, alt_tags=False):
                """QKV projections + V-natural for pair t, as filler units.

                alt_tags alternates the psum slot between the w and y tags so
                consecutive units pipeline (y slot is free whenever final(t-1)
                is not in flight)."""
                qT = qkv_pool.tile([128, M], mdt, tag="qT", name="qT")
                kT = qkv_pool.tile([128, M], mdt, tag="kT", name="kT")
                vT = qkv_pool.tile([128, M], tdt, tag="vT", name="vT")
                v_nat = vnat_pool.tile([128, NT * 130], mdt, tag="vn", name="v_nat")
                tags = ["w", "y"] if alt_tags else ["w", "w"]
                nunit = [0]

                def proj(pi, w_sb, dest, mh):
                    def f():
                        tag = tags[nunit[0] % 2]
                        nunit[0] += 1
                        with nc.named_scope(f"qkv_p{t}"):
                            ps = ps_pool.tile(
                                [128, 512], f32, tag=tag,
                                bufs=cfg["w_bufs" if tag == "w" else "y_bufs"],
                                name="psQKV",
                            )
                            nc.tensor.matmul(
                                ps[:],
                                w_sb[:],
                                xT_all[:, t * M + mh * 512 : t * M + (mh + 1) * 512],
                                start=True,
                                stop=True,
                            )
                            nc.vector.tensor_scalar_add(
                                dest[:, mh * 512 : (mh + 1) * 512],
                                ps[:],
                                bias_sb[:, pi : pi + 1],
                            )
                    return f

                def vtrans(g):
                    def f():
                        tag = tags[nunit[0] % 2]
                        nunit[0] += 1
                        with nc.named_scope(f"qkv_p{t}"):
                            pst = ps_pool.tile(
                                [128, 512], tdt, tag=tag,
                                bufs=cfg["w_bufs" if tag == "w" else "y_bufs"],
                                name="psVT",
                            )
                            for j in range(4):
                                nt = 4 * g + j
                                nc.tensor.transpose(
                                    pst[:, j * 128 : (j + 1) * 128],
                                    vT[:, nt * 128 : (nt + 1) * 128],
                                    identity[:],
                                )
                            vdst = v_nat[:, g * 520 : (g + 1) * 520].rearrange(
                                "p (n c) -> p n c", c=130
                            )
                            vsrc = pst[:].rearrange("p (n c) -> p n c", c=128)
                            nc.vector.tensor_copy(vdst[:, :, 0:64], vsrc[:, :, 0:64])
                            nc.vector.tensor_copy(
                                vdst[:, :, 65:129], vsrc[:, :, 64:128]
                            )
                            nc.gpsimd.memset(vdst[:, :, 64:65].bitcast(mdt), 1.0)
                            nc.gpsimd.memset(vdst[:, :, 129:130].bitcast(mdt), 1.0)
                    return f

                units = []
                # order: q(mh0), k(mh0) first so scores of mh0 can start after
                # two units; then v(mh0), vtrans(0) (AV of nt0-3), then mh1.
                units.append(proj(0, wq_sb, qT, 0))
                units.append(proj(1, wk_sb, kT, 0))
                units.append(proj(2, wv_sb, vT, 0))
                units.append(vtrans(0))
                units.append(proj(0, wq_sb, qT, 1))
                units.append(proj(1, wk_sb, kT, 1))
                units.append(proj(2, wv_sb, vT, 1))
                units.append(vtrans(1))
                return (qT, kT, v_nat), units

            def final_units(t):
                """Output-projection rows for pair t (j = 128t..128t+127)."""
                y_sb = ysb_pool.tile([128, 1024], mdt, tag="y", name="y_sb")
                psY_box = [None]

                def mms(dh, mts):
                    def f():
                        with nc.named_scope(f"final_p{t}"):
                            if psY_box[0] is None:
                                psY_box[0] = ps_pool.tile(
                                    [128, 512], f32, tag="y", bufs=cfg["y_bufs"],
                                    name="psY",
                                )
                            psY = psY_box[0]
                            for mt in mts:
                                nc.tensor.matmul(
                                    psY[:],
                                    PT_all[:, mt * D + t * 128 : mt * D + (t + 1) * 128],
                                    woT_all[
                                        :, mt * D + dh * 512 : mt * D + (dh + 1) * 512
                                    ],
                                    start=(mt == 0),
                                    stop=(mt == NT - 1),
                                )
                            if mts[-1] == NT - 1:
                                nc.vector.tensor_copy(
                                    y_sb[:, dh * 512 : (dh + 1) * 512], psY[:]
                                )
                                psY_box[0] = None
                                nc.sync.dma_start(
                                    y_ap[
                                        t * 128 : (t + 1) * 128,
                                        dh * 512 : (dh + 1) * 512,
                                    ],
                                    y_sb[:, dh * 512 : (dh + 1) * 512],
                                )
                    return f

                return [mms(dh, [2 * c, 2 * c + 1]) for dh in range(2) for c in range(4)]

            def emit_attn_mh(t, mh, qT, kT, v_nat, u_sbs):
                """Scores + exp + AV for both heads of pair t, half mh.

                Emitted at artificially old priority so that whenever a score
                matmul becomes ready it pops ahead of any filler work -- this
                keeps the even/odd row-tile pair adjacent in the PE queue and
                thus hardware-concurrent."""
                with tc.high_priority(offset=500000), \
                     nc.named_scope(f"attn_p{t}_m{mh}"):
                    expS = exps_pool.tile(
                        [128, NT * 1024], mdt, tag="es", name="expS"
                    )
                    psU = [
                        ps_pool.tile(
                            [65, 512], f32, tag="u", bufs=cfg["u_bufs"], name="psU"
                        )
                        for _ in range(2)
                    ]
                    for nt in range(NT):
                        psS = ps_pool.tile(
                            [128, 1024], f32, tag="s", bufs=cfg["s_bufs"],
                            name="psS",
                        )
                        # Even/odd head score matmuls: K=64 row-tiles at
                        # partitions 0:64 / 64:128 -> concurrent on PE.
                        for hh in range(2):
                            part = hh * 64
                            nc.tensor.matmul(
                                psS[:, hh * 512 : (hh + 1) * 512],
                                qT[part : part + 64, nt * 128 : (nt + 1) * 128],
                                kT[part : part + 64, mh * 512 : (mh + 1) * 512],
                                start=True,
                                stop=True,
                            )
                        # One ACT instruction for both heads' scores.
                        nc.scalar.activation(
                            expS[:, nt * 1024 : (nt + 1) * 1024],
                            psS[:],
                            Exp,
                            scale=1.0 / 32.0,
                        )
                        # AV accumulation for both heads.
                        for hh in range(2):
                            o = nt * 130 + hh * 65
                            nc.tensor.matmul(
                                psU[hh][:],
                                v_nat[:, o : o + 65],
                                expS[:, nt * 1024 + hh * 512 : nt * 1024 + (hh + 1) * 512],
                                start=(nt == 0),
                                stop=(nt == NT - 1),
                            )
                    for hh in range(2):
                        nc.vector.tensor_copy(
                            u_sbs[hh][:, mh * 512 : (mh + 1) * 512],
                            psU[hh][:],
                        )

            def norm_units(t, mh, u_sbs):
                """Transpose+normalize U (m-half mh) into PT_all, per head."""
                def one(hh):
                    def f():
                        h = 2 * t + hh
                        u_sb = u_sbs[hh]
                        with nc.named_scope(f"norm_h{h}"):
                            rec = rec_pool.tile([128, 4], f32, tag="r", name="rec")
                            pstU = ps_pool.tile(
                                [128, 512], tdt, tag="w", bufs=cfg["w_bufs"],
                                name="pstU",
                            )
                            for j in range(4):
                                mt = 4 * mh + j
                                nc.tensor.transpose(
                                    pstU[:, j * 128 : j * 128 + 65],
                                    u_sb[:, mt * 128 : (mt + 1) * 128],
                                    identity[:65, :65],
                                )
                            nc.vector.tensor_copy(
                                rec[:, 0:4],
                                pstU[:]
                                .rearrange("p (n c) -> p n c", c=128)[:, :, 64:65]
                                .rearrange("p n c -> p (n c)"),
                            )
                            nc.vector.reciprocal(rec[:], rec[:])
                            for j in range(4):
                                mt = 4 * mh + j
                                nc.vector.tensor_scalar_mul(
                                    PT_all[:, mt * D + h * 64 : mt * D + h * 64 + 64],
                                    pstU[:, j * 128 : j * 128 + 64],
                                    rec[:, j : j + 1],
                                )
                    return f
                return [one(0), one(1)]

            # ---- pair loop ----
            def interleave(*lists):
                out = []
                idx = [0] * len(lists)
                while any(idx[i] < len(lists[i]) for i in range(len(lists))):
                    for i, l in enumerate(lists):
                        if idx[i] < len(l):
                            out.append(l[idx[i]])
                            idx[i] += 1
                return out

            cur, qkv_u = push_qkv(0, alt_tags=True)
            for f in qkv_u:
                f()  # qkv(0) on the critical path, emit immediately
            emit_woT_loads()
            prev_norm1 = []   # norm(t-1, 1) units
            for t in range(8):
                u_sbs = [
                    usb_pool.tile([65, M], tdt, tag="u", name="u_sb")
                    for _ in range(2)
                ]
                qT, kT, v_nat = cur
                fin_u = final_units(t - 1) if t > 0 else []
                if t + 1 < 8:
                    cur, qkv_u = push_qkv(t + 1, alt_tags=(t == 0))
                else:
                    qkv_u = []
                fillers.extend(prev_norm1)
                fillers.extend(interleave(qkv_u, fin_u))
                emit_attn_mh(t, 0, qT, kT, v_nat, u_sbs)
                pop_fillers(6)
                fillers.extend(norm_units(t, 0, u_sbs))
                emit_attn_mh(t, 1, qT, kT, v_nat, u_sbs)
                flush_fillers()  # qkv(t+1) must finish before attn(t+1,0)
                prev_norm1 = norm_units(t, 1, u_sbs)
            for f in prev_norm1:
                f()
            for f in final_units(7):
                f()

    nc.compile()
    return nc


def _get_compiled(mode):
    if mode not in _compiled:
        _compiled[mode] = _build(mode)
    return _compiled[mode]


def _prep_inputs(mode, x, Wq, bq, Wk, bk, Wv, bv, Wo, bo):
    np_mdt = np.float16

    def blockdiag_lhsT(W):
        out = np.zeros((128, 128), np.float32)
        out[:64, :64] = W.T
        out[64:, 64:] = W.T
        return out.astype(np_mdt)

    wq_bd = blockdiag_lhsT(Wq)
    wk_bd = blockdiag_lhsT(Wk)
    wv_bd = blockdiag_lhsT(Wv)
    bias = np.stack(
        [np.concatenate([b, b]) for b in (bq, bk, bv)], axis=1
    ).astype(np.float32)  # [128, 3]
    woT = np.ascontiguousarray(Wo.T).astype(np_mdt)
    xT = np.ascontiguousarray(np.transpose(x, (0, 2, 1))).astype(np_mdt)  # [B,D,M]
    in_maps = [
        {
            "xT": xT[b],
            "woT": woT,
            "wq": wq_bd,
            "wk": wk_bd,
            "wv": wv_bd,
            "bias": bias,
        }
        for b in range(B)
    ]
    return in_maps


def run(inputs, trace=False, trace_kwargs=None, mode=DTYPE_MODE):
    """Run on HW; returns (full_output, BassKernelResults)."""
    from concourse.bass_utils import run_bass_kernel_spmd

    inputs = {k: np.asarray(v) for k, v in inputs.items()}
    nc = _get_compiled(mode)
    in_maps = _prep_inputs(
        mode,
        inputs["x"],
        inputs["Wq"], inputs["bq"],
        inputs["Wk"], inputs["bk"],
        inputs["Wv"], inputs["bv"],
        inputs["Wo"], inputs["bo"],
    )
    kw = dict(trace_kwargs or {})
    res = run_bass_kernel_spmd(nc, in_maps, list(range(B)), trace=trace, **kw)
    out = np.empty((B, M, D), np.float32)
    out5 = out.reshape(B, 2, 8, 64, D)  # [bo, s, b, d, Do]
    for b in range(B):
        Y = np.asarray(res.results[b]["y"], np.float32)  # [1024(j), 1024(Do)]
        out5[:, :, b] = Y.reshape(8, 2, 64, D)
    out += np.asarray(inputs["bo"], np.float32)[None, None, :]
    return out, res


def kernel(**inputs):
    out, _ = run(inputs)
    return out


# revision 17
# speedup vs baseline: 1.4677x; 1.4677x over previous
"""Multi-head attention Trainium2 kernel (Bass/Tile), data-parallel over batch.

Problem shapes (hardcoded): x [8, 1024, 1024] fp32, 16 heads x 64 dim,
shared per-head projections Wq/Wk/Wv [64, 64], output proj Wo [1024, 1024].

Reference math (note quirks):
  xh = x reshaped to [h, b, m, d]
  Q/K/V = xh @ W{q,k,v}.T + b
  scores = einsum('hbmd,hbnd->hbmn', K, Q) / sqrt(1024)   (K @ Q^T!)
  A = softmax(scores, axis=-1)
  out = (A @ V) transposed (0,1,3,2) then .reshape(b, m, D) @ Wo.T + bo

Per-core plan (core b handles batch b, no collectives):
  - host prepares xT = x[b].T, blockdiag lhsT weights for 2-head packed
    projections, WoT = Wo.T
  - QT/KT/VT [64*16, m] via blockdiag [128,128] matmuls
  - scores: per (pair, m-half, n-block) ONE psS tile [128, e-512|o-512];
    the two K=64 matmuls (partitions 0:64 / 64:128) row-tile and run
    CONCURRENTLY on the PE; one ACT instruction exps both.
  - U[65, m] = [V | ones].T @ expS  -> row 64 = softmax denominator
  - PE-transpose U -> [m, 65], normalize cols by 1/denominator -> P.T
    (norm done per m-half right after its attention phase)
  - Y rows for the pair's heads = P.T chunk.T @ WoT (bo added on host)
  - QKV of pair t+1 and output-projection of pair t-1 are emitted as
    fine-grained "fillers" between attention steps so the scheduler can
    use PE slack without ever blocking the score->exp pipeline.
"""

import os

import numpy as np

B = 8
M = 1024
D = 1024
NT = 8  # 128-row tiles in M / D

DTYPE_MODE = os.environ.get("KERNEL_DTYPE", "f16")

DEFAULT_CFG = dict(
    s_bufs=2,             # score psum tiles [128,1024] (2 banks each)
    w_bufs=1,             # weights-path psum tile [128,512]
    y_bufs=1,             # final-path psum tile [128,512]
    u_bufs=2,             # AV accumulator psum tiles [65,512]
    qkv_bufs=3,           # qT/kT/vT sbuf pipelining depth
    vnat_bufs=3,
    usb_bufs=4,
    ysb_bufs=3,
    es_bufs=3,            # expS [128, NT*1024] fp16 tiles
)

_compiled = {}


def _build(mode, cfg=None):
    import concourse.bacc as bacc
    import concourse.mybir as mybir
    import concourse.tile as tile
    from concourse.masks import make_identity

    cfg = dict(DEFAULT_CFG, **(cfg or {}))
    f32 = mybir.dt.float32
    mdt = mybir.dt.float16
    tdt = mdt  # transpose-path dtype
    Exp = mybir.ActivationFunctionType.Exp

    nc = bacc.Bacc("TRN2", target_bir_lowering=False, debug=False, num_devices=B)

    xT_ap = nc.dram_tensor("xT", [D, M], mdt, kind="ExternalInput").ap()
    woT_ap = nc.dram_tensor("woT", [D, D], mdt, kind="ExternalInput").ap()
    wq_ap = nc.dram_tensor("wq", [128, 128], mdt, kind="ExternalInput").ap()
    wk_ap = nc.dram_tensor("wk", [128, 128], mdt, kind="ExternalInput").ap()
    wv_ap = nc.dram_tensor("wv", [128, 128], mdt, kind="ExternalInput").ap()
    bias_ap = nc.dram_tensor("bias", [128, 3], f32, kind="ExternalInput").ap()
    y_ap = nc.dram_tensor("y", [D, M], mdt, kind="ExternalOutput").ap()

    with tile.TileContext(nc) as tc:
        with (
            tc.tile_pool(name="persist", bufs=1) as persist,
            tc.tile_pool(name="qkv", bufs=cfg["qkv_bufs"]) as qkv_pool,
            tc.tile_pool(name="vnat", bufs=cfg.get("vnat_bufs", 2)) as vnat_pool,
            tc.tile_pool(name="exps", bufs=cfg["es_bufs"]) as exps_pool,
            tc.tile_pool(name="usb", bufs=cfg.get("usb_bufs", 3)) as usb_pool,
            tc.tile_pool(name="ysb", bufs=cfg.get("ysb_bufs", 2)) as ysb_pool,
            tc.tile_pool(name="rec", bufs=8) as rec_pool,
            tc.tile_pool(name="ps", bufs=1, space="PSUM") as ps_pool,
        ):
            # ---- persistent tiles ----
            xT_all = persist.tile([128, NT * M], mdt)  # tile t at cols t*M
            woT_all = persist.tile([128, NT * D], mdt)
            PT_all = persist.tile([128, NT * D], mdt)  # [m-local, mt*D + h*64+d]
            wq_sb = persist.tile([128, 128], mdt)
            wk_sb = persist.tile([128, 128], mdt)
            wv_sb = persist.tile([128, 128], mdt)
            bias_sb = persist.tile([128, 3], f32)
            identity = persist.tile([128, 128], tdt)
            dummy = persist.tile([128, 1], f32)

            # Preload the exp table set while input DMAs are in flight.
            nc.scalar.activation(dummy[:], dummy[:], Exp, scale=1.0)

            with nc.named_scope("loads"):
                nc.sync.dma_start(wq_sb[:], wq_ap[:])
                nc.sync.dma_start(wk_sb[:], wk_ap[:])
                nc.sync.dma_start(
                    xT_all[:, 0:M], xT_ap[0:128, :]
                )
                nc.sync.dma_start(wv_sb[:], wv_ap[:])
                nc.sync.dma_start(bias_sb[:], bias_ap[:])
                # PE warm-up: ~4us of dummy matmuls on garbage data during the
                # DMA wait lifts the HAM clock gate before real work arrives.
                warm = persist.tile([128, 512], mdt)
                nc.gpsimd.memset(warm[:], 1.0)
                for _ in range(9):
                    psWarm = ps_pool.tile(
                        [128, 1024], f32, tag="s", bufs=cfg["s_bufs"], name="psWarm"
                    )
                    nc.tensor.matmul(
                        psWarm[:, 0:512], warm[:, 0:128], warm[:], start=True,
                        stop=True,
                    )
                make_identity(nc, identity[:])
                for t in range(1, NT):
                    nc.sync.dma_start(
                        xT_all[:, t * M : (t + 1) * M],
                        xT_ap[t * 128 : (t + 1) * 128, :],
                    )

            def emit_woT_loads():
                with nc.named_scope("loads_wo"):
                    for t in range(NT):
                        nc.sync.dma_start(
                            woT_all[:, t * D : (t + 1) * D],
                            woT_ap[t * 128 : (t + 1) * 128, :],
                        )

            # ---- filler queue: list of 0-arg callables, emitted between
            # attention steps so they sit at lower scheduler priority ----
            fillers = []

            def pop_fillers(k):
                for _ in range(min(k, len(fillers))):
                    fillers.pop(0)()

            def flush_fillers():
                pop_fillers(len(fillers))

            def push_qkv(t, alt_tags=False):
                """QKV projections + V-natural for pair t, as filler units.

                alt_tags alternates the psum slot between the w and y tags so
                consecutive units pipeline (y slot is free whenever final(t-1)
                is not in flight)."""
                qT = qkv_pool.tile([128, M], mdt, tag="qT", name="qT")
                kT = qkv_pool.tile([128, M], mdt, tag="kT", name="kT")
                vT = qkv_pool.tile([128, M], tdt, tag="vT", name="vT")
                v_nat = vnat_pool.tile([128, NT * 130], mdt, tag="vn", name="v_nat")
                tags = ["w", "y"] if alt_tags else ["w", "w"]
                nunit = [0]

                def proj(pi, w_sb, dest, mh):
                    def f():
                        tag = tags[nunit[0] % 2]
                        nunit[0] += 1
                        with nc.named_scope(f"qkv_p{t}"):
                            ps = ps_pool.tile(
                                [128, 512], f32, tag=tag,
                                bufs=cfg["w_bufs" if tag == "w" else "y_bufs"],
                                name="psQKV",
                            )
                            nc.tensor.matmul(
                                ps[:],
                                w_sb[:],
                                xT_all[:, t * M + mh * 512 : t * M + (mh + 1) * 512],
                                start=True,
                                stop=True,
                            )
                            nc.vector.tensor_scalar_add(
                                dest[:, mh * 512 : (mh + 1) * 512],
                                ps[:],
                                bias_sb[:, pi : pi + 1],
                            )
                    return f

                def vtrans(g):
                    def f():
                        tag = tags[nunit[0] % 2]
                        nunit[0] += 1
                        with nc.named_scope(f"qkv_p{t}"):
                            pst = ps_pool.tile(
                                [128, 512], tdt, tag=tag,
                                bufs=cfg["w_bufs" if tag == "w" else "y_bufs"],
                                name="psVT",
                            )
                            for j in range(4):
                                nt = 4 * g + j
                                nc.tensor.transpose(
                                    pst[:, j * 128 : (j + 1) * 128],
                                    vT[:, nt * 128 : (nt + 1) * 128],
                                    identity[:],
                                )
                            vdst = v_nat[:, g * 520 : (g + 1) * 520].rearrange(
                                "p (n c) -> p n c", c=130
                            )
                            vsrc = pst[:].rearrange("p (n c) -> p n c", c=128)
                            nc.vector.tensor_copy(vdst[:, :, 0:64], vsrc[:, :, 0:64])
                            nc.vector.tensor_copy(
                                vdst[:, :, 65:129], vsrc[:, :, 64:128]
                            )
                            nc.gpsimd.memset(vdst[:, :, 64:65].bitcast(mdt), 1.0)
                            nc.gpsimd.memset(vdst[:, :, 129:130].bitcast(mdt), 1.0)
                    return f

                units = []
                # order: q(mh0), k(mh0) first so scores of mh0 can start after
                # two units; then v(mh0), vtrans(0) (AV of nt0-3), then mh1.
                units.append(proj(0, wq_sb, qT, 0))
                units.append(proj(1, wk_sb, kT, 0))
                units.append(proj(2, wv_sb, vT, 0))
                units.append(vtrans(0))
                units.append(proj(0, wq_sb, qT, 1))
                units.append(proj(1, wk_sb, kT, 1))
                units.append(proj(2, wv_sb, vT, 1))
                units.append(vtrans(1))
                return (qT, kT, v_nat), units

            def final_units(t, dh_tags=("y", "y")):
                """Output-projection rows for pair t (j = 128t..128t+127)."""
                y_sb = ysb_pool.tile([128, 1024], mdt, tag="y", name="y_sb")
                psY_box = [None, None]

                def mms(dh, mts):
                    def f():
                        with nc.named_scope(f"final_p{t}"):
                            if psY_box[dh] is None:
                                tag = dh_tags[dh]
                                psY_box[dh] = ps_pool.tile(
                                    [128, 512], f32, tag=tag,
                                    bufs=cfg["w_bufs" if tag == "w" else "y_bufs"],
                                    name="psY",
                                )
                            psY = psY_box[dh]
                            for mt in mts:
                                nc.tensor.matmul(
                                    psY[:],
                                    PT_all[:, mt * D + t * 128 : mt * D + (t + 1) * 128],
                                    woT_all[
                                        :, mt * D + dh * 512 : mt * D + (dh + 1) * 512
                                    ],
                                    start=(mt == 0),
                                    stop=(mt == NT - 1),
                                )
                            if mts[-1] == NT - 1:
                                nc.vector.tensor_copy(
                                    y_sb[:, dh * 512 : (dh + 1) * 512], psY[:]
                                )
                                psY_box[dh] = None
                                nc.sync.dma_start(
                                    y_ap[
                                        t * 128 : (t + 1) * 128,
                                        dh * 512 : (dh + 1) * 512,
                                    ],
                                    y_sb[:, dh * 512 : (dh + 1) * 512],
                                )
                    return f

                return [mms(dh, [2 * c, 2 * c + 1]) for dh in range(2) for c in range(4)]

            def emit_attn_mh(t, mh, qT, kT, v_nat, u_sbs):
                """Scores + exp + AV for both heads of pair t, half mh.

                The score matmuls and the exp are emitted at artificially old
                priority: whenever a score matmul becomes ready it pops ahead
                of any filler/AV work, keeping the even/odd row-tile pair
                adjacent in the PE queue (hardware-concurrent) and the ACT
                engine maximally supplied."""
                with nc.named_scope(f"attn_p{t}_m{mh}"):
                    expS = exps_pool.tile(
                        [128, NT * 1024], mdt, tag="es", name="expS"
                    )
                    psU = [
                        ps_pool.tile(
                            [65, 512], f32, tag="u", bufs=cfg["u_bufs"], name="psU"
                        )
                        for _ in range(2)
                    ]
                    for nt in range(NT):
                        with tc.high_priority(offset=500000):
                            psS = ps_pool.tile(
                                [128, 1024], f32, tag="s", bufs=cfg["s_bufs"],
                                name="psS",
                            )
                            # Even/odd head score matmuls: K=64 row-tiles at
                            # partitions 0:64 / 64:128 -> concurrent on PE.
                            for hh in range(2):
                                part = hh * 64
                                nc.tensor.matmul(
                                    psS[:, hh * 512 : (hh + 1) * 512],
                                    qT[part : part + 64, nt * 128 : (nt + 1) * 128],
                                    kT[part : part + 64, mh * 512 : (mh + 1) * 512],
                                    start=True,
                                    stop=True,
                                )
                            # One ACT instruction for both heads' scores.
                            nc.scalar.activation(
                                expS[:, nt * 1024 : (nt + 1) * 1024],
                                psS[:],
                                Exp,
                                scale=1.0 / 32.0,
                            )
                        # AV accumulation for both heads (normal priority).
                        for hh in range(2):
                            o = nt * 130 + hh * 65
                            nc.tensor.matmul(
                                psU[hh][:],
                                v_nat[:, o : o + 65],
                                expS[:, nt * 1024 + hh * 512 : nt * 1024 + (hh + 1) * 512],
                                start=(nt == 0),
                                stop=(nt == NT - 1),
                            )
                        pop_fillers(1 if nt % 2 == 0 else 2)
                    for hh in range(2):
                        nc.vector.tensor_copy(
                            u_sbs[hh][:, mh * 512 : (mh + 1) * 512],
                            psU[hh][:],
                        )

            def norm_units(t, mh, u_sbs):
                """Transpose+normalize U (m-half mh) into PT_all, per head."""
                def one(hh):
                    def f():
                        h = 2 * t + hh
                        u_sb = u_sbs[hh]
                        with nc.named_scope(f"norm_h{h}"):
                            rec = rec_pool.tile([128, 4], f32, tag="r", name="rec")
                            pstU = ps_pool.tile(
                                [128, 512], tdt, tag="w", bufs=cfg["w_bufs"],
                                name="pstU",
                            )
                            for j in range(4):
                                mt = 4 * mh + j
                                nc.tensor.transpose(
                                    pstU[:, j * 128 : j * 128 + 65],
                                    u_sb[:, mt * 128 : (mt + 1) * 128],
                                    identity[:65, :65],
                                )
                            nc.vector.tensor_copy(
                                rec[:, 0:4],
                                pstU[:]
                                .rearrange("p (n c) -> p n c", c=128)[:, :, 64:65]
                                .rearrange("p n c -> p (n c)"),
                            )
                            nc.vector.reciprocal(rec[:], rec[:])
                            for j in range(4):
                                mt = 4 * mh + j
                                nc.vector.tensor_scalar_mul(
                                    PT_all[:, mt * D + h * 64 : mt * D + h * 64 + 64],
                                    pstU[:, j * 128 : j * 128 + 64],
                                    rec[:, j : j + 1],
                                )
                    return f
                return [one(0), one(1)]

            # ---- pair loop ----
            def interleave(*lists):
                out = []
                idx = [0] * len(lists)
                while any(idx[i] < len(lists[i]) for i in range(len(lists))):
                    for i, l in enumerate(lists):
                        if idx[i] < len(l):
                            out.append(l[idx[i]])
                            idx[i] += 1
                return out

            cur, qkv_u = push_qkv(0, alt_tags=True)
            for f in qkv_u:
                f()  # qkv(0) on the critical path, emit immediately
            emit_woT_loads()
            prev_norm1 = []   # norm(t-1, 1) units
            for t in range(8):
                u_sbs = [
                    usb_pool.tile([65, M], tdt, tag="u", name="u_sb")
                    for _ in range(2)
                ]
                qT, kT, v_nat = cur
                fin_u = final_units(t - 1) if t > 0 else []
                if t + 1 < 8:
                    cur, qkv_u = push_qkv(t + 1, alt_tags=(t == 0))
                else:
                    qkv_u = []
                fillers.extend(prev_norm1)
                fillers.extend(interleave(qkv_u, fin_u))
                emit_attn_mh(t, 0, qT, kT, v_nat, u_sbs)
                fillers.extend(norm_units(t, 0, u_sbs))
                emit_attn_mh(t, 1, qT, kT, v_nat, u_sbs)
                flush_fillers()  # qkv(t+1) must finish before attn(t+1,0)
                prev_norm1 = norm_units(t, 1, u_sbs)
            # Tail: run final(7)'s two dh chains in parallel psum slots; the
            # mt 0-3 matmuls only need norm(7,0), so they overlap norm(7,1).
            # norm(7,1) must be emitted before the dh1 chain allocates the
            # shared w slot (else slot-wait cycle).
            fin7 = final_units(7, dh_tags=("y", "w"))
            fin7[0]()  # dh0 mt01 (y slot, needs only norm(7,0))
            fin7[1]()  # dh0 mt23
            for f in prev_norm1:  # norm(7,1): DVE-heavy, uses w slot briefly
                f()
            fin7[4]()  # dh1 mt01 (w slot)
            fin7[5]()  # dh1 mt23
            fin7[2]()
            fin7[6]()
            fin7[3]()
            fin7[7]()

    nc.compile()
    return nc


def _get_compiled(mode):
    if mode not in _compiled:
        _compiled[mode] = _build(mode)
    return _compiled[mode]


def _prep_inputs(mode, x, Wq, bq, Wk, bk, Wv, bv, Wo, bo):
    np_mdt = np.float16

    def blockdiag_lhsT(W):
        out = np.zeros((128, 128), np.float32)
        out[:64, :64] = W.T
        out[64:, 64:] = W.T
        return out.astype(np_mdt)

    wq_bd = blockdiag_lhsT(Wq)
    wk_bd = blockdiag_lhsT(Wk)
    wv_bd = blockdiag_lhsT(Wv)
    bias = np.stack(
        [np.concatenate([b, b]) for b in (bq, bk, bv)], axis=1
    ).astype(np.float32)  # [128, 3]
    woT = np.ascontiguousarray(Wo.T).astype(np_mdt)
    xT = np.ascontiguousarray(np.transpose(x, (0, 2, 1))).astype(np_mdt)  # [B,D,M]
    in_maps = [
        {
            "xT": xT[b],
            "woT": woT,
            "wq": wq_bd,
            "wk": wk_bd,
            "wv": wv_bd,
            "bias": bias,
        }
        for b in range(B)
    ]
    return in_maps


def run(inputs, trace=False, trace_kwargs=None, mode=DTYPE_MODE):
    """Run on HW; returns (full_output, BassKernelResults)."""
    from concourse.bass_utils import run_bass_kernel_spmd

    inputs = {k: np.asarray(v) for k, v in inputs.items()}
    nc = _get_compiled(mode)
    in_maps = _prep_inputs(
        mode,
        inputs["x"],
        inputs["Wq"], inputs["bq"],
        inputs["Wk"], inputs["bk"],
        inputs["Wv"], inputs["bv"],
        inputs["Wo"], inputs["bo"],
    )
    kw = dict(trace_kwargs or {})
    res = run_bass_kernel_spmd(nc, in_maps, list(range(B)), trace=trace, **kw)
    out = np.empty((B, M, D), np.float32)
    out5 = out.reshape(B, 2, 8, 64, D)  # [bo, s, b, d, Do]
    for b in range(B):
        Y = np.asarray(res.results[b]["y"], np.float32)  # [1024(j), 1024(Do)]
        out5[:, :, b] = Y.reshape(8, 2, 64, D)
    out += np.asarray(inputs["bo"], np.float32)[None, None, :]
    return out, res


def kernel(**inputs):
    out, _ = run(inputs)
    return out
